# revision 15
# baseline (speedup 1.0000x reference)
"""MultiHeadAttention (B=4, S=2048, D=1024, H=16, causal + key mask) on 8 trn2 cores.

Sharding: Megatron-style tensor parallel over heads. Each core owns 2 heads:
column slices of Wq/Wk/Wv (D x 128), the matching row slice of Wp (128 x D).
Each core computes a partial output y_c = attn_c @ Wp_c; host sums the 8
partials and adds bp.

v3 (bf16 + engine rebalance + latency fixes):
  - All matmuls bf16 (fp32r measured ~2x slower per column on HW).
  - Scores for the 2 heads run CONCURRENTLY as PE row-tiles (contraction 64,
    tile_position (0,0)/(64,0) auto-derived from base partitions).
  - Causal mask: multiplicative 0/1 bf16 mask AFTER exp on DVE (2x tier);
    key-mask stays the exp bias. Diagonal blocks restrict q to [128j, 512).
  - Exp owns ScalarE during attention; projection evictions (bias fused) on
    ScalarE fill its idle time since projections of batch b+1 are interleaved
    into the attention of batch b (keeps PE warm, hides exp latency).
  - PV psum is evicted to SBUF immediately at accumulation stop so the psum
    bank recycles without waiting on the normalize chain; the reciprocal uses
    reciprocal_approx_fast (the exact DVE reciprocal costs ~3us per row).
  - Scores are emitted one k-block ahead of PV so the in-order PE queue never
    head-of-line blocks on the current block's exp.
  - PV keeps the ones-column trick (lhsT = [V | 1], row 64 of the PV psum
    accumulates the softmax denominator).
"""

import numpy as np

P = 128
B, S, D, H = 4, 2048, 1024, 16
HD = D // H  # 64
NCORES = 8
HPC = H // NCORES  # 2 heads per core
BS = B * S  # 8192
NB = S // P  # 16 k-blocks per batch
NG = S // 512  # 4 q-groups per batch

_CACHE = {}


def _build_nc():
    import concourse.mybir as mybir
    from concourse import bacc
    from concourse.tile import TileContext
    from concourse.masks import make_identity
    from contextlib import ExitStack

    f32 = mybir.dt.float32
    bf16 = mybir.dt.bfloat16
    AF = mybir.ActivationFunctionType

    nc = bacc.Bacc("TRN2", target_bir_lowering=False, debug=False,
                   num_devices=NCORES)

    xT_d = nc.dram_tensor("xT", [D, BS], bf16, kind="ExternalInput").ap()
    wq_d = nc.dram_tensor("wq", [D, P], bf16, kind="ExternalInput").ap()
    wk_d = nc.dram_tensor("wk", [D, P], bf16, kind="ExternalInput").ap()
    wv_d = nc.dram_tensor("wv", [D, P], bf16, kind="ExternalInput").ap()
    bq_d = nc.dram_tensor("bq", [P, 1], f32, kind="ExternalInput").ap()
    bk_d = nc.dram_tensor("bk", [P, 1], f32, kind="ExternalInput").ap()
    bv_d = nc.dram_tensor("bv", [P, 1], f32, kind="ExternalInput").ap()
    wp_d = nc.dram_tensor("wp", [P, D], bf16, kind="ExternalInput").ap()
    mb_d = nc.dram_tensor("maskb", [P, B * NB], f32, kind="ExternalInput").ap()
    cm_d = nc.dram_tensor("cmask", [P, 4, HPC, 512], bf16,
                          kind="ExternalInput").ap()
    yp_d = nc.dram_tensor("yp", [BS, D], bf16, kind="ExternalOutput").ap()

    xT_r = xT_d.rearrange("(o p) n -> p o n", p=P)  # [128, 8, 8192]
    KD = D // P  # 8 contraction chunks

    with TileContext(nc) as tc:
        with ExitStack() as ctx:
            consts = ctx.enter_context(tc.tile_pool(name="consts", bufs=1))
            big = ctx.enter_context(tc.tile_pool(name="big", bufs=1))
            xpool = ctx.enter_context(tc.tile_pool(name="xpool", bufs=2))
            vtpool = ctx.enter_context(tc.tile_pool(name="vtpool", bufs=2))
            ptpool = ctx.enter_context(tc.tile_pool(name="ptpool", bufs=3))
            pvspool = ctx.enter_context(tc.tile_pool(name="pvs", bufs=2))
            npool = ctx.enter_context(tc.tile_pool(name="npool", bufs=2))
            ypool = ctx.enter_context(tc.tile_pool(name="ypool", bufs=3))
            psum = ctx.enter_context(
                tc.tile_pool(name="psum", bufs=2, space="PSUM"))
            sc2pool = ctx.enter_context(
                tc.tile_pool(name="sc2pool", bufs=2, space="PSUM"))
            pvpool = ctx.enter_context(
                tc.tile_pool(name="pvpool", bufs=2, space="PSUM"))

            # ---- constants ----
            wq_sb = consts.tile([P, KD, P], bf16, tag="wq")
            wk_sb = consts.tile([P, KD, P], bf16, tag="wk")
            wv_sb = consts.tile([P, KD, P], bf16, tag="wv")
            nc.sync.dma_start(wq_sb[:], wq_d.rearrange("(o p) m -> p o m", p=P))
            nc.sync.dma_start(wk_sb[:], wk_d.rearrange("(o p) m -> p o m", p=P))
            nc.sync.dma_start(wv_sb[:], wv_d.rearrange("(o p) m -> p o m", p=P))
            wp_sb = consts.tile([P, D], bf16, tag="wp")
            nc.sync.dma_start(wp_sb[:], wp_d)
            bq_sb = consts.tile([P, 1], f32, tag="bq")
            bk_sb = consts.tile([P, 1], f32, tag="bk")
            bv_sb = consts.tile([P, 1], f32, tag="bv")
            nc.sync.dma_start(bq_sb[:], bq_d)
            nc.sync.dma_start(bk_sb[:], bk_d)
            nc.sync.dma_start(bv_sb[:], bv_d)
            mb_sb = consts.tile([P, B * NB], f32, tag="mb")
            nc.sync.dma_start(mb_sb[:], mb_d)
            # multiplicative causal masks, [p, j, head, q'] 0/1 bf16
            cm_sb = consts.tile([P, 4, HPC, 512], bf16, tag="cm")
            nc.sync.dma_start(cm_sb[:], cm_d)
            ident = consts.tile([P, P], bf16, tag="ident")
            make_identity(nc, ident[:])

            # ---- persistent activations (all bf16) ----
            qt_sb = big.tile([P, B, S], bf16, tag="qt")  # Q^T
            kt_sb = big.tile([P, B, S], bf16, tag="kt")  # K^T
            # V in [s, hd] layout + ones col: [p=s%128, b, sblock, h, 65]
            v_sb = big.tile([P, B, NB, HPC, HD + 1], bf16, tag="v")
            at_sb = big.tile([P, B, S], bf16, tag="at")  # attn^T (normalized)
            nc.vector.memset(v_sb[:, :, :, :, HD], 1.0)

            # ---- projections for one 512-row chunk of x ----
            def proj_chunk(c):
                b, sc = divmod(c, NG)
                xt = xpool.tile([P, KD, 512], bf16, tag="xt")
                nc.sync.dma_start(xt[:], xT_r[:, :, c * 512:(c + 1) * 512])
                ssl = slice(sc * 512, (sc + 1) * 512)
                for which in range(3):
                    w_sb = (wq_sb, wk_sb, wv_sb)[which]
                    ps = psum.tile([P, 512], f32, tag="ps")
                    for o in range(KD):
                        nc.tensor.matmul(
                            ps[:], lhsT=w_sb[:, o, :], rhs=xt[:, o, :],
                            start=(o == 0), stop=(o == KD - 1))
                    if which == 0:
                        nc.scalar.activation(qt_sb[:, b, ssl], ps[:],
                                             AF.Identity, bias=bq_sb[:])
                    elif which == 1:
                        nc.scalar.activation(kt_sb[:, b, ssl], ps[:],
                                             AF.Identity, bias=bk_sb[:])
                    else:
                        vt = vtpool.tile([P, 512], bf16, tag="vt")
                        nc.scalar.activation(vt[:], ps[:], AF.Identity,
                                             bias=bv_sb[:])
                        for t in range(4):
                            # shares the "ps" slots (pools size per tag)
                            trp = psum.tile([P, P], bf16, tag="ps")
                            nc.tensor.transpose(
                                trp[:], vt[:, t * P:(t + 1) * P], ident[:])
                            sb_i = sc * 4 + t
                            nc.vector.tensor_copy(
                                v_sb[:, b, sb_i, 0, 0:HD], trp[:, 0:HD])
                            nc.vector.tensor_copy(
                                v_sb[:, b, sb_i, 1, 0:HD],
                                trp[:, HD:2 * HD])

            # ---- output projection for one (b, g) q-group ----
            def outproj(b, g):
                for qc in range(4):
                    q0 = g * 512 + qc * P
                    r0 = b * S + q0
                    y_sb = ypool.tile([P, 2, 512], bf16, tag="y",
                                      name=f"y_{b}_{g}_{qc}")
                    for half in range(2):
                        yp_ps = psum.tile([P, 512], f32, tag="ps",
                                          name=f"yps_{b}_{g}_{qc}_{half}")
                        nc.tensor.matmul(
                            yp_ps[:],
                            lhsT=at_sb[:, b, q0:q0 + P],
                            rhs=wp_sb[:, half * 512:(half + 1) * 512],
                            start=True, stop=True)
                        nc.vector.tensor_copy(y_sb[:, half, :], yp_ps[:])
                    nc.sync.dma_start(
                        yp_d[r0:r0 + P, :],
                        y_sb[:].rearrange("p a n -> p (a n)"))

            # ---- attention for one (b, g) q-group ----
            def attn_group(b, g, pending):
                gsl = slice(g * 512, (g + 1) * 512)
                nkb = 4 * (g + 1)
                pvs = [pvpool.tile([P, 512], f32, tag="pv",
                                   name=f"pv_{b}_{g}_{h}")
                       for h in range(HPC)]

                def scores(kb):
                    j = kb - 4 * g
                    # diagonal blocks: q < 128*j is fully masked
                    qo = 128 * max(j, 0)
                    sc2 = sc2pool.tile([P, HPC, 512], f32, tag="sc2",
                                       name=f"sc2_{b}_{g}_{kb}")
                    for h in range(HPC):
                        hsl = slice(h * HD, (h + 1) * HD)
                        nc.tensor.matmul(
                            sc2[:, h, qo:512],
                            lhsT=kt_sb[hsl, b, kb * P:(kb + 1) * P],
                            rhs=qt_sb[hsl, b, g * 512 + qo:(g + 1) * 512],
                            start=True, stop=(j < 0))
                    if j >= 0:
                        # additive causal mask via identity-matmul accumulate:
                        # keeps masking on the PE, so PV depends only on exp
                        for h in range(HPC):
                            nc.tensor.matmul(
                                sc2[:, h, qo:512], lhsT=ident[:],
                                rhs=cm_sb[:, j, h, qo:512],
                                start=False, stop=True)
                    return sc2, j, qo

                cur = scores(0)
                for kb in range(nkb):
                    nxt = scores(kb + 1) if kb + 1 < nkb else None
                    sc2, j, qo = cur
                    col = b * NB + kb
                    pt = ptpool.tile([P, HPC, 512], bf16, tag="pt")
                    if qo == 0:
                        nc.scalar.activation(pt[:], sc2[:], AF.Exp,
                                             bias=mb_sb[:, col:col + 1])
                    else:
                        nc.scalar.activation(pt[:, :, qo:512],
                                             sc2[:, :, qo:512], AF.Exp,
                                             bias=mb_sb[:, col:col + 1])
                    for h in range(HPC):
                        nc.tensor.matmul(
                            pvs[h][0:HD + 1, qo:512],
                            lhsT=v_sb[:, b, kb, h, :],
                            rhs=pt[:, h, qo:512],
                            start=(kb == 0), stop=(kb == nkb - 1))
                    cur = nxt
                # evict PV psums immediately so the banks recycle without
                # waiting on the normalize chain
                pvs_sb = pvspool.tile([P, HPC, 512], f32, tag="pvs")
                for h in range(HPC):
                    nc.vector.tensor_copy(pvs_sb[0:HD + 1, h, :],
                                          pvs[h][0:HD + 1, :])
                # lag 2 while projections interleave; lag 1 on the last batch
                # to shorten the end-of-kernel dependency tail
                lag = 2 if b < B - 1 else 1
                if len(pending) >= lag:
                    outproj(*pending.pop(0))
                pending.append((b, g))
                # ---- normalize: 1/denom (row 64) -> broadcast -> mul ----
                # reciprocal_approx_fast misbehaves on single-partition
                # slices -- run it over the full tile (unused rows discarded)
                rcp = npool.tile([P, HPC, 512], f32, tag="rcp")
                nc.vector.reciprocal_approx_fast(rcp[:], pvs_sb[:])
                # broadcast + muls + shift on GpSimd and its DMA queue: keeps
                # this latency-tolerant chain out of DVE's in-order queue and
                # off the sync DMA queue (which carries the big y writes)
                dbc = npool.tile([HD, HPC, 512], f32, tag="dbc")
                for h in range(HPC):
                    nc.gpsimd.dma_start(
                        dbc[:, h, :],
                        rcp[HD:HD + 1, h, None, :].to_broadcast((1, HD, 512)))
                nc.gpsimd.tensor_mul(at_sb[0:HD, b, gsl],
                                     pvs_sb[0:HD, 0, :], dbc[:, 0, :])
                tmp = npool.tile([HD, 512], bf16, tag="tmp")
                nc.gpsimd.tensor_mul(tmp[:], pvs_sb[0:HD, 1, :], dbc[:, 1, :])
                nc.gpsimd.dma_start(at_sb[HD:2 * HD, b, gsl], tmp[:])

            # ---- schedule: batch-0 projections, then attention(b) with
            # batch-(b+1) projections interleaved per q-group ----
            for c in range(NG):
                proj_chunk(c)
            pending = []
            for b in range(B):
                for g in range(NG):
                    attn_group(b, g, pending)
                    if b + 1 < B:
                        proj_chunk(NG * (b + 1) + g)
            for pg in pending:
                outproj(*pg)

    nc.compile()
    return nc


def _get_nc():
    if "nc" not in _CACHE:
        _CACHE["nc"] = _build_nc()
    return _CACHE["nc"]


def make_in_maps(x, attention_mask, Wq, bq, Wk, bk, Wv, bv, Wp, bp):
    """Host-side sharding: build the 8 per-core device input maps."""
    import ml_dtypes
    bf16 = ml_dtypes.bfloat16
    x = np.asarray(x, dtype=np.float32)
    scale = np.float32(1.0 / np.sqrt(HD))
    xT = np.ascontiguousarray(x.reshape(BS, D).T.astype(bf16))  # [D, BS]
    mb = (np.asarray(attention_mask).astype(np.float32) - 1.0) * np.float32(1e9)
    mb = np.ascontiguousarray(
        mb.reshape(B, NB, P).transpose(2, 0, 1).reshape(P, B * NB))
    # multiplicative causal masks: 1 where 128*j + p <= q', else 0;
    # duplicated for the two heads: [128, 4, 2, 512]
    pp = np.arange(P)[:, None]
    ff = np.arange(512)[None, :]
    cm = np.stack(
        [np.where(P * j + pp <= ff, 0.0, -1e9).astype(bf16)
         for j in range(4)], axis=1)  # [128, 4, 512]
    cm = np.ascontiguousarray(
        np.broadcast_to(cm[:, :, None, :], (P, 4, HPC, 512)))

    Wq = (np.asarray(Wq, np.float32) * scale).astype(bf16)
    bq = np.asarray(bq, np.float32) * scale
    Wk = np.asarray(Wk, np.float32).astype(bf16)
    bk = np.asarray(bk, np.float32)
    Wv = np.asarray(Wv, np.float32).astype(bf16)
    bv = np.asarray(bv, np.float32)
    Wp = np.asarray(Wp, np.float32).astype(bf16)

    in_maps = []
    for c in range(NCORES):
        cs = slice(c * P, (c + 1) * P)
        in_maps.append({
            "xT": xT,
            "wq": np.ascontiguousarray(Wq[:, cs]),
            "wk": np.ascontiguousarray(Wk[:, cs]),
            "wv": np.ascontiguousarray(Wv[:, cs]),
            "bq": np.ascontiguousarray(bq[cs].reshape(P, 1)),
            "bk": np.ascontiguousarray(bk[cs].reshape(P, 1)),
            "bv": np.ascontiguousarray(bv[cs].reshape(P, 1)),
            "wp": np.ascontiguousarray(Wp[cs, :]),
            "maskb": mb,
            "cmask": cm,
        })
    return in_maps


def run(inputs, trace=False, tmpdir=None):
    """Compile (cached) + run on 8 cores. Returns (output, BassKernelResults)."""
    from concourse import bass_utils
    nc = _get_nc()
    in_maps = make_in_maps(**inputs)
    kwargs = {}
    if trace:
        kwargs = dict(trace=True, tmpdir=tmpdir)
    res = bass_utils.run_bass_kernel_spmd(
        nc, in_maps, core_ids=list(range(NCORES)), **kwargs)
    acc = np.zeros((BS, D), dtype=np.float32)
    for r in res.results:
        acc += r["yp"].astype(np.float32)
    out = acc + np.asarray(inputs["bp"], np.float32)[None, :]
    return out.reshape(B, S, D), res


def kernel(**inputs) -> np.ndarray:
    out, _ = run(inputs, trace=False)
    return out
